# revision 13
# baseline (speedup 1.0000x reference)
"""BasicUformerLayer (2-block Swin/Uformer stage) Trainium2 Bass kernel.

Sharding: 8 cores = (batch b in {0,1}) x (row-quarter j in {0..3}).
Core receives xin rows [32j, 32j+40) (40-row slice), computes block 0 for
all 5 bands (x1 rows [32j+4, 32j+36) kept), block 1 for the 4 shifted
bands fully inside that range, and outputs y rows [32j+4, 32j+36); the
host stitches quarters. Each shifted band is computed by exactly one
core (no duplicated block-1 bands; block-0 redundancy is 2 half-bands).

Layouts on core:
 - tokens processed in 128-token tiles = 2 windows x 64 tokens, partition
   p = w*64 + r*8 + c (window-major); groups of 4 tiles (512 tokens).
 - ALL matmuls bf16 (FWL weight loads).
 - window attention with transposed scores S^T[m,n]; exp via ACT,
   rel-pos bias multiplicative (exp(bias) precomputed), denominator via
   ones-column in V.
 - LayerNorm rstd via DVE-only Newton rsqrt; ACT runs only exp/gelu.
 - block0 -> block1 residual x1 in DRAM (bf16); NO barrier at the block
   boundary: each block-1 band's x1 rows are stored >=6 groups earlier.
 - software pipeline per group g: prep_dma(g+1) (x DMAs + LN1 stats) at
   the top of main(g); prep_xform(g+1) (Newton + LN applies + XnT
   transposes) after do_back(1); qkt(g+1) after the backs; front(g+1,0)
   after fc1 -> PE never drains at group boundaries (HAM stays warm).
 - PSUM banks: scores/AV 4 (concurrent tile-position writers must hit
   different banks unless partition-disjoint!), transposes 2 (+V),
   QK/fc1/proj/fc2 share 2.
"""

import numpy as np
import ml_dtypes

import concourse.bass as bass
from concourse import bacc
import concourse.mybir as mybir
import concourse.tile as tile
from concourse.bass_utils import run_bass_kernel_spmd
from concourse.masks import make_identity

WS = 8
HEADS = 16
HD = 32
C = 512
HID = 2048
H = W = 128
B = 2
EPS = 1e-5
F32, BF16, U32 = mybir.dt.float32, mybir.dt.bfloat16, mybir.dt.uint32
AF = mybir.ActivationFunctionType
OP = mybir.AluOpType


def _rel_pos_index(ws):
    coords = np.stack(np.meshgrid(np.arange(ws), np.arange(ws), indexing='ij')).reshape(2, -1)
    rel = (coords[:, :, None] - coords[:, None, :]).transpose(1, 2, 0)
    rel[:, :, 0] += ws - 1
    rel[:, :, 1] += ws - 1
    rel[:, :, 0] *= 2 * ws - 1
    return rel.sum(-1)


def _shift_attn_mask(H_, W_, ws, shift):
    img = np.zeros((H_, W_))
    cnt = 0
    for hs in (slice(0, -ws), slice(-ws, -shift), slice(-shift, None)):
        for wsl in (slice(0, -ws), slice(-ws, -shift), slice(-shift, None)):
            img[hs, wsl] = cnt
            cnt += 1
    mw = img.reshape(H_ // ws, ws, W_ // ws, ws).transpose(0, 2, 1, 3).reshape(-1, ws * ws)
    diff = mw[:, None, :] - mw[:, :, None]
    return np.where(diff != 0, -100.0, 0.0).astype(np.float32)  # (nW, N, N)


# ---------------------------------------------------------------- kernel build

def _rsqrt4(nc, pools, mvg, magic):
    """Batch 1/sqrt(var+eps) for 4 tiles on DVE only (no ACT table).
    mvg [128,4,2] f32 (mean,var); returns f32 view [128,4] of rstd."""
    uf = pools["small2"].tile([128, 4], F32, name="uf", tag="uf")
    nc.vector.tensor_scalar(uf, mvg[:, :, 1], EPS, None, OP.add)
    iu = pools["small2"].tile([128, 4], U32, name="iu", tag="iu")
    nc.vector.tensor_scalar(iu, uf[:].bitcast(U32), 1, None, OP.logical_shift_right)
    nc.vector.tensor_tensor(iu, magic[:], iu, OP.subtract)
    y = iu[:].bitcast(F32)
    t = pools["small2"].tile([128, 4], F32, name="nt", tag="nt")
    for _ in range(2):
        nc.vector.tensor_tensor(t, y, y, OP.mult)
        nc.vector.tensor_tensor(t, t, uf, OP.mult)
        nc.vector.tensor_scalar(t, t, -0.5, 1.5, OP.mult, OP.add)
        nc.vector.tensor_tensor(y, y, t, OP.mult)
    return y


def _transpose4(nc, pools, src_bf, dst, dst_k_slice, identity, on_act=False):
    """src [128,512] bf16 -> dst[:, k, dst_k_slice] = src chunkT (4 PE transposes).
    on_act=True alternates ACT/DVE evacuation so neither engine paces the PE."""
    for k in range(4):
        pst = pools["ps_tp"].tile([128, 128], BF16, name="tp", tag="tp")
        nc.tensor.transpose(pst, src_bf[:, 128 * k:128 * (k + 1)], identity)
        if on_act and k % 2 == 0:
            nc.scalar.activation(dst[:, k, dst_k_slice], pst[:], AF.Copy)
        else:
            nc.vector.tensor_copy(dst[:, k, dst_k_slice], pst)


def _attn_front(nc, pools, XnT, QT, KT, toff, qkvw, expb):
    """V + scores + exp + bias-mult for one 128-token tile; returns (Vp, attn)."""
    tslice = slice(toff, toff + 128)
    Vp = pools["act"].tile([128, HEADS, 64], BF16, name="Vp", tag="Vp")
    nc.vector.memset(Vp[:, :, 32:33], 1.0)
    psv = pools["ps_tp"].tile([128, 512], F32, name="psv", tag="tp")
    for k in range(4):
        nc.tensor.matmul(psv, XnT[:, k, tslice], qkvw[:, k, 1024:1536],
                         start=(k == 0), stop=(k == 3))
    nc.vector.tensor_copy(Vp[:, :, 0:32],
                          psv[:].rearrange("p (h e) -> p h e", h=HEADS))
    sb = []
    for i in range(4):
        t_ = pools["ps_s"].tile([128, 4, 64], F32, name=f"s{i}", tag=f"sa{i}")
        sb.append(t_)
    for g in range(4):
        for i in range(4):
            for w in range(2):
                nc.tensor.matmul(
                    sb[i][64 * w:64 * w + 64, g, :],
                    KT[32 * i:32 * i + 32, g, toff + 64 * w:toff + 64 * w + 64],
                    QT[32 * i:32 * i + 32, g, toff + 64 * w:toff + 64 * w + 64],
                    start=True, stop=True, tile_position=(32 * i, 64 * w))
    attn = pools["act"].tile([128, HEADS, 64], BF16, name="attn", tag="attn")
    for i in range(4):
        # exp then bias-mult per head-group: AV of group i can start as
        # soon as its own exp+mult land (not after all four)
        nc.scalar.activation(attn[:, i:HEADS:4, :], sb[i][:], AF.Exp)
        nc.vector.tensor_tensor(attn[:, i:HEADS:4, :], attn[:, i:HEADS:4, :],
                                expb[:, i:HEADS:4, :], OP.mult)
    return Vp, attn


def _attn_back(nc, pools, Vp, attn, O):
    """AV (+denominator) + normalize into O [128,512] bf16."""
    absb = pools["act"].tile([128, 4, 4, 33], F32, name="absb", tag="absb")
    for i in range(4):
        abp = pools["ps_s"].tile([128, 4, 64], F32, name=f"a{i}", tag=f"sa{i}")
        for hh in (i, 4 + i, 8 + i, 12 + i):
            for w in range(2):
                nc.tensor.matmul(
                    abp[64 * w:64 * w + 64, hh // 4, 0:33],
                    attn[64 * w:64 * w + 64, hh, :],
                    Vp[64 * w:64 * w + 64, hh, 0:33],
                    start=True, stop=True, tile_position=(64 * w, 64 * w))
        # evacuate the bank with one copy so the next tile's score MMs can
        # claim it ~1us earlier than the recip/mult chain would allow
        nc.vector.tensor_copy(absb[:, i], abp[:, :, 0:33])
    rden = pools["act"].tile([128, 4, 4], F32, name="rden", tag="rden")
    Ov = O[:].rearrange("p (h e) -> p h e", h=HEADS)
    for i in range(4):
        nc.vector.reciprocal(rden[:, i], absb[:, i, :, 32])
        nc.vector.tensor_tensor(
            Ov[:, i:HEADS:4, :], absb[:, i, :, 0:32],
            rden[:, i, :, None].to_broadcast((128, 4, 32)),
            OP.mult)


def build(act=AF.Gelu):
    """Build the per-core Bacc program (same NEFF on all 8 cores)."""
    nc = bacc.Bacc("TRN2", target_bir_lowering=False, debug=False)

    xin_d = nc.dram_tensor("xin", (40, 128, C), F32, kind="ExternalInput")
    qkvw_d = nc.dram_tensor("qkvw", (2, C, 3 * C), BF16, kind="ExternalInput")
    qkvb_d = nc.dram_tensor("qkvb", (2, 128, 12), F32, kind="ExternalInput")
    projw_d = nc.dram_tensor("projw", (2, C, C), BF16, kind="ExternalInput")
    fc1w_d = nc.dram_tensor("fc1w", (2, C, HID), BF16, kind="ExternalInput")
    fc1b_d = nc.dram_tensor("fc1b", (2, 128, 16), F32, kind="ExternalInput")
    fc2w_d = nc.dram_tensor("fc2w", (2, HID, C), BF16, kind="ExternalInput")
    expb0_d = nc.dram_tensor("expb0", (128, HEADS * 64), BF16, kind="ExternalInput")
    expb1_d = nc.dram_tensor("expb1", (4, 2, 128, HEADS * 64), BF16, kind="ExternalInput")
    y_d = nc.dram_tensor("y", (32, 128, C), F32, kind="ExternalOutput")

    with tile.TileContext(nc) as tc:
        pools = {}
        import contextlib
        ctx = contextlib.ExitStack()
        with ctx:
            pools["w"] = ctx.enter_context(tc.tile_pool(name="w", bufs=1))
            pools["w2"] = ctx.enter_context(tc.tile_pool(name="w2", bufs=2))
            pools["const"] = ctx.enter_context(tc.tile_pool(name="const", bufs=1))
            pools["act"] = ctx.enter_context(tc.tile_pool(name="act", bufs=2))
            pools["x2t"] = ctx.enter_context(tc.tile_pool(name="x2t", bufs=1))
            pools["x"] = ctx.enter_context(tc.tile_pool(name="x", bufs=10))
            pools["xn"] = ctx.enter_context(tc.tile_pool(name="xn", bufs=4))
            pools["xm"] = ctx.enter_context(tc.tile_pool(name="xm", bufs=5))
            pools["h1"] = ctx.enter_context(tc.tile_pool(name="h1", bufs=1))
            pools["dram"] = ctx.enter_context(tc.tile_pool(name="dram", bufs=1, space="DRAM"))
            pools["eb"] = ctx.enter_context(tc.tile_pool(name="eb", bufs=2))
            pools["small"] = ctx.enter_context(tc.tile_pool(name="small", bufs=4))
            pools["small2"] = ctx.enter_context(tc.tile_pool(name="small2", bufs=3))
            pools["ps_tp"] = ctx.enter_context(tc.tile_pool(name="ps_tp", bufs=2, space="PSUM"))
            pools["ps_qk"] = ctx.enter_context(tc.tile_pool(name="ps_qk", bufs=2, space="PSUM"))
            pools["ps_v"] = pools["ps_qk"]  # proj/fc2 share the 2 qk banks
            pools["ps_s"] = ctx.enter_context(tc.tile_pool(name="ps_s", bufs=1, space="PSUM"))

            identity_f32 = pools["const"].tile([128, 128], F32, name="identity_f32")
            make_identity(nc, identity_f32)
            identity_b = pools["const"].tile([128, 128], BF16, name="identity")
            nc.vector.tensor_copy(identity_b, identity_f32)
            identity = identity_b[:]
            magic = pools["const"].tile([128, 4], U32, name="magic")
            nc.vector._memset_packed(magic[:], 0x5f3759df)
            expb0 = pools["const"].tile([128, HEADS, 64], BF16, name="expb0")
            nc.sync.dma_start(expb0, expb0_d[:].rearrange("p (h n) -> p h n", h=HEADS))

            # block0 -> block1 residual in DRAM (bf16). No barrier: stores of
            # the rows any block-1 band reads complete >=6 groups earlier,
            # and loads are on the gpsimd DMA queue (stores: sync queue).
            x1 = pools["dram"].tile([40, 128, C], BF16, name="x1")

            # flattened group list across both blocks
            all_groups = []
            for d in (0, 1):
                tiles = ([(k, t) for k in range(5) for t in range(8)] if d == 0
                         else [(i, t) for i in range(4) for t in range(8)])
                for gi in range(0, len(tiles), 4):
                    all_groups.append((d, tiles[gi:gi + 4]))
            NG = len(all_groups)

            wq = {}   # per-d qkv weights (loaded at prep_dma crossing)
            wm = {}   # per-d proj/fc1 weights (prefetched 2 groups early)
            wf2 = {}  # per-d fc2 weights (loaded at main crossing)
            ebs = {}  # per-stripe exp-bias tiles (block 1)
            state = {}  # gidx -> dict

            def load_wq(d):
                qkvw = pools["w2"].tile([128, 4, 3 * C], BF16, name="qkvw", tag="qkvw")
                nc.sync.dma_start(qkvw, qkvw_d[d].rearrange("(ko ki) n -> ki ko n", ki=128))
                qkvb = pools["w2"].tile([128, 12], F32, name="qkvb", tag="qkvb")
                nc.sync.dma_start(qkvb, qkvb_d[d])
                wq[d] = (qkvw, qkvb)

            def load_wm_early(d):
                projw = pools["w2"].tile([128, 4, C], BF16, name="projw", tag="projw")
                nc.sync.dma_start(projw, projw_d[d].rearrange("(ko ki) n -> ki ko n", ki=128))
                fc1w = pools["w2"].tile([128, 4, HID], BF16, name="fc1w", tag="fc1w")
                nc.sync.dma_start(fc1w, fc1w_d[d].rearrange("(ko ki) n -> ki ko n", ki=128))
                fc1b = pools["w2"].tile([128, 16], F32, name="fc1b", tag="fc1b")
                nc.sync.dma_start(fc1b, fc1b_d[d])
                wm[d] = (projw, fc1w, fc1b)

            def load_wm_late(d):
                fc2w = pools["w"].tile([128, 16, C], BF16, name="fc2w", tag="fc2w")
                nc.sync.dma_start(fc2w, fc2w_d[d].rearrange("(ko ki) n -> ki ko n", ki=128))
                wf2[d] = fc2w

            def prep_dma(gidx):
                """x DMAs (gpsimd queue) + LN1 stats for all 4 tiles of gidx."""
                d, gts = all_groups[gidx]
                if d not in wq:
                    load_wq(d)
                st_XnT = pools["act"].tile([128, 4, 512], BF16, name="XnT", tag="XnT")
                st_mvg = pools["small"].tile([128, 4, 2], F32, name="mvg", tag="mvg")
                state[gidx] = {"XnT": st_XnT, "xs": [], "mvg": st_mvg,
                               "xns": [], "fronts": []}
                st = state[gidx]
                for ti, (kk, t) in enumerate(gts):
                    if d == 1 and t == 0:
                        eb_u = pools["eb"].tile([128, HEADS, 64], BF16, name="eb_u", tag="eb_u")
                        nc.sync.dma_start(
                            eb_u, expb1_d[kk, 0].rearrange("p (h n) -> p h n", h=HEADS))
                        eb_m = pools["eb"].tile([128, HEADS, 64], BF16, name="eb_m", tag="eb_m")
                        nc.sync.dma_start(
                            eb_m, expb1_d[kk, 1].rearrange("p (h n) -> p h n", h=HEADS))
                        ebs[kk] = (eb_u, eb_m)
                    if d == 0:
                        xt = pools["x"].tile([128, C], F32, name="xt", tag="xt")
                        for w in range(2):
                            nc.sync.dma_start(
                                xt[64 * w:64 * w + 64],
                                xin_d[8 * kk:8 * kk + 8, 16 * t + 8 * w:16 * t + 8 * w + 8, :])
                    else:
                        xt = pools["x"].tile([128, C], BF16, name="xtb", tag="xt")
                        r0 = 8 * kk + 4
                        if t < 7:
                            for w in range(2):
                                nc.sync.dma_start(
                                    xt[64 * w:64 * w + 64],
                                    x1[r0:r0 + 8, 16 * t + 4 + 8 * w:16 * t + 12 + 8 * w, :])
                        else:
                            # win14 row-major; win15 col-major (p=64+8c+r)
                            nc.sync.dma_start(xt[0:64], x1[r0:r0 + 8, 116:124, :])
                            nc.sync.dma_start(
                                xt[64:96],
                                x1[r0:r0 + 8, 124:128, :].rearrange("r c e -> c r e"))
                            nc.sync.dma_start(
                                xt[96:128],
                                x1[r0:r0 + 8, 0:4, :].rearrange("r c e -> c r e"))
                    st["xs"].append(xt)
                    stats = pools["small"].tile([128, 6], F32, name="lnstats", tag="lnstats")
                    nc.vector.bn_stats(stats, xt[:])
                    nc.vector.bn_aggr(st["mvg"][:, ti], stats)

            def prep_xform(gidx):
                """Newton rsqrt + LN1 applies + XnT transposes for gidx."""
                st = state[gidx]
                rstd = _rsqrt4(nc, pools, st["mvg"], magic)
                for tj in range(4):
                    xn = pools["xn"].tile([128, C], BF16, name="xn", tag="xn")
                    nc.vector.tensor_scalar(xn, st["xs"][tj][:], st["mvg"][:, tj, 0:1],
                                            rstd[:, tj:tj + 1], OP.subtract, OP.mult)
                    st["xns"].append(xn)
                    _transpose4(nc, pools, xn, st["XnT"],
                                slice(128 * tj, 128 * tj + 128), identity, on_act=True)

            def qkt_group(gidx):
                d, _ = all_groups[gidx]
                qkvw, qkvb = wq[d]
                XnT = state[gidx]["XnT"]
                QT = pools["act"].tile([128, 4, 512], BF16, name="QT", tag="QT")
                KT = pools["act"].tile([128, 4, 512], BF16, name="KT", tag="KT")
                for qk, dst in ((0, QT), (1, KT)):
                    for g in range(4):
                        ps = pools["ps_qk"].tile([128, 512], F32, name="psqk", tag="qk")
                        for k in range(4):
                            nc.tensor.matmul(
                                ps, qkvw[:, k, 512 * qk + 128 * g: 512 * qk + 128 * (g + 1)],
                                XnT[:, k], start=(k == 0), stop=(k == 3))
                        nc.scalar.activation(dst[:, g], ps[:], AF.Identity,
                                             bias=qkvb[:, 4 * qk + g: 4 * qk + g + 1])
                state[gidx]["qkkt"] = (QT, KT)
                return QT, KT

            def do_front(gidx, ti):
                d, gts = all_groups[gidx]
                kk, t = gts[ti]
                st = state[gidx]
                QT, KT = st["qkkt"]
                qkvw, _ = wq[d]
                if d == 0:
                    expb = expb0
                else:
                    expb = ebs[kk][1 if t == 7 else 0]
                st["fronts"].append(
                    _attn_front(nc, pools, st["XnT"], QT, KT, 128 * ti, qkvw, expb))

            def main_group(gidx):
                d, gts = all_groups[gidx]
                if d not in wm:
                    load_wm_early(d)
                if d not in wf2:
                    load_wm_late(d)
                # prefetch proj/fc1 of the next block 2 groups before the
                # crossing (their w2 slots are free; DMA overlaps compute)
                if gidx + 2 < NG:
                    d2 = all_groups[gidx + 2][0]
                    if d2 not in wm:
                        load_wm_early(d2)
                projw, fc1w, fc1b = wm[d]
                fc2w = wf2[d]
                # next group's x loads + LN1 stats first (DMA queue decoupled
                # from this group's stores; slots sized to never block here)
                if gidx + 1 < NG:
                    prep_dma(gidx + 1)
                st = state[gidx]
                xs = st["xs"]
                mvg2 = pools["small"].tile([128, 4, 2], F32, name="mvg2", tag="mvg2")
                xmids = []

                def do_back(ti):
                    Vp, attn = st["fronts"][ti]
                    O = pools["act"].tile([128, C], BF16, name="O", tag="O")
                    _attn_back(nc, pools, Vp, attn, O)
                    OT = pools["act"].tile([128, 4, 128], BF16, name="OT", tag="OT")
                    _transpose4(nc, pools, O, OT, slice(0, 128), identity)
                    psp = pools["ps_v"].tile([128, C], F32, name="psproj", tag="qk")
                    for k in range(4):
                        nc.tensor.matmul(psp, OT[:, k], projw[:, k],
                                         start=(k == 0), stop=(k == 3))
                    xmid = pools["xm"].tile([128, C], F32, name="xmid", tag="xmid")
                    nc.vector.tensor_add(xmid, psp, xs[ti])
                    xmids.append(xmid)
                    stats = pools["small"].tile([128, 6], F32, name="lnstats", tag="lnstats")
                    nc.vector.bn_stats(stats, xmid[:])
                    nc.vector.bn_aggr(mvg2[:, ti], stats)

                for ti in range(4):
                    if ti >= len(st["fronts"]):
                        do_front(gidx, ti)
                    do_back(ti)
                    # next group's LN1 transform right after back(1): its DVE
                    # chain lands mid-group, transposes/QK fill the PE queue
                    if ti == 1 and gidx + 1 < NG:
                        prep_xform(gidx + 1)
                # next group's QK (fills PE while LN2 applies run on DVE)
                if gidx + 1 < NG:
                    qkt_group(gidx + 1)
                rstd2 = _rsqrt4(nc, pools, mvg2, magic)
                Xn2T = pools["x2t"].tile([128, 4, 512], BF16, name="Xn2T", tag="Xn2T")
                for ti in range(4):
                    xn2 = pools["xn"].tile([128, C], BF16, name="xn2", tag="xn")
                    nc.vector.tensor_scalar(xn2, xmids[ti][:], mvg2[:, ti, 0:1],
                                            rstd2[:, ti:ti + 1], OP.subtract, OP.mult)
                    _transpose4(nc, pools, xn2, Xn2T,
                                slice(128 * ti, 128 * ti + 128), identity)
                # fc1 + act -> hT bf16 [128, 16, 512]
                hT = pools["h1"].tile([128, 16, 512], BF16, name="hT", tag="hT")
                for hc in range(16):
                    psf = pools["ps_qk"].tile([128, 512], F32, name="psfc1", tag="qk")
                    for k in range(4):
                        nc.tensor.matmul(psf, fc1w[:, k, 128 * hc:128 * (hc + 1)],
                                         Xn2T[:, k], start=(k == 0), stop=(k == 3))
                    nc.scalar.activation(hT[:, hc], psf, act, bias=fc1b[:, hc:hc + 1])
                # pre-emit next group's first attention front: its V/scores
                # run during fc2 and the softmax exp overlaps fc2's stream,
                # so AV is ready the moment fc2 drains
                if gidx + 1 < NG:
                    do_front(gidx + 1, 0)
                # fc2 + residual + store
                for ti, (kk, t) in enumerate(gts):
                    psf2 = pools["ps_v"].tile([128, C], F32, name="psfc2", tag="qk")
                    for hc in range(16):
                        nc.tensor.matmul(psf2, hT[:, hc, 128 * ti:128 * (ti + 1)],
                                         fc2w[:, hc], start=(hc == 0), stop=(hc == 15))
                    if d == 0:
                        ytb = pools["act"].tile([128, C], BF16, name="ytb", tag="ytb")
                        nc.vector.tensor_add(ytb, psf2, xmids[ti])
                        rlo, rhi = (4, 8) if kk == 0 else ((0, 4) if kk == 4 else (0, 8))
                        for w in range(2):
                            nc.sync.dma_start(
                                x1[8 * kk + rlo:8 * kk + rhi,
                                   16 * t + 8 * w:16 * t + 8 * w + 8, :],
                                ytb[64 * w + 8 * rlo:64 * w + 8 * rhi])
                    else:
                        yt = pools["act"].tile([128, C], F32, name="yt", tag="yt")
                        nc.vector.tensor_add(yt, psf2, xmids[ti])
                        rlo, rhi = 0, 8
                        ylo, yhi = 8 * kk, 8 * kk + 8
                        if t < 7:
                            for w in range(2):
                                nc.sync.dma_start(
                                    y_d[ylo:yhi, 16 * t + 4 + 8 * w:16 * t + 12 + 8 * w, :],
                                    yt[64 * w + 8 * rlo:64 * w + 8 * rhi])
                        else:
                            nc.sync.dma_start(y_d[ylo:yhi, 116:124, :], yt[8 * rlo:8 * rhi])
                            for c in range(4):
                                nc.sync.dma_start(y_d[ylo:yhi, 124 + c, :],
                                                  yt[64 + 8 * c + rlo:64 + 8 * c + rhi])
                                nc.sync.dma_start(y_d[ylo:yhi, c, :],
                                                  yt[96 + 8 * c + rlo:96 + 8 * c + rhi])
                del state[gidx]

            # prologue: prep + QKT of group 0
            prep_dma(0)
            prep_xform(0)
            qkt_group(0)
            for g in range(NG):
                main_group(g)
    nc.finalize()
    return nc


# ---------------------------------------------------------------- host wrapper

_CACHED = {}


def _prep(inputs):
    x = np.asarray(inputs["x"], np.float32).reshape(B, H, W, C)
    qkv_w = np.asarray(inputs["qkv_w"], np.float32)
    qkv_b = np.asarray(inputs["qkv_b"], np.float32)
    n1w = np.asarray(inputs["norm1_w"], np.float32)
    n1b = np.asarray(inputs["norm1_b"], np.float32)
    n2w = np.asarray(inputs["norm2_w"], np.float32)
    n2b = np.asarray(inputs["norm2_b"], np.float32)
    proj_w = np.asarray(inputs["proj_w"], np.float32)
    proj_b = np.asarray(inputs["proj_b"], np.float32)
    fc1_w = np.asarray(inputs["fc1_w"], np.float32)
    fc1_b = np.asarray(inputs["fc1_b"], np.float32)
    fc2_w = np.asarray(inputs["fc2_w"], np.float32)
    fc2_b = np.asarray(inputs["fc2_b"], np.float32)
    rpb = np.asarray(inputs["rpb"], np.float32)

    assert not proj_b.any() and not fc2_b.any(), "nonzero proj/fc2 bias unsupported"

    sc = HD ** -0.5
    qkvw_f = np.empty((2, C, 3 * C), np.float32)
    qkvb_f = np.empty((2, 3 * C), np.float32)
    fc1w_f = np.empty((2, C, HID), np.float32)
    fc1b_f = np.empty((2, HID), np.float32)
    for d in range(2):
        wf = qkv_w[d] * n1w[d][:, None]
        bf = qkv_b[d] + n1b[d] @ qkv_w[d]
        wf[:, :C] *= sc
        bf[:C] *= sc
        qkvw_f[d] = wf
        qkvb_f[d] = bf
        fc1w_f[d] = fc1_w[d] * n2w[d][:, None]
        fc1b_f[d] = fc1_b[d] + n2b[d] @ fc1_w[d]
    assert not qkvb_f[:, 1024:].any(), "nonzero V bias unsupported"
    qkvb_qk = qkvb_f[:, :1024].reshape(2, 8, 128).transpose(0, 2, 1)  # (2,128,8)
    qkvb_in = np.concatenate([qkvb_qk, np.zeros((2, 128, 4), np.float32)], axis=2)
    fc1b_in = fc1b_f.reshape(2, 16, 128).transpose(0, 2, 1)  # (2,128,16)

    rel = _rel_pos_index(WS)
    biasT = np.empty((2, HEADS, 64, 64), np.float32)  # [d, h, m, n]
    for d in range(2):
        bd = rpb[d][rel]  # (n, m, heads)
        biasT[d] = bd.transpose(2, 1, 0)  # h, m, n
    expb0 = np.exp(biasT[0]).transpose(1, 0, 2).reshape(64, HEADS * 64)
    expb0 = np.concatenate([expb0, expb0], 0)  # (128, 1024) both windows

    mask = _shift_attn_mask(H, W, WS, WS // 2)  # (256, 64, 64) [win, n, m]
    return (x, qkvw_f, qkvb_in, proj_w, fc1w_f, fc1b_in, fc2_w,
            expb0, biasT, mask)


def _expb1_core(j, biasT, mask):
    """(4, 2, 128, 1024) per-core block-1 exp-bias tables."""
    out = np.empty((4, 2, 128, HEADS * 64), np.float32)
    for i in range(4):
        a = (4 * j + i) % 16
        for sel in range(2):
            bs = (14, 15) if sel == 1 else (0, 1)
            perm = np.arange(64).reshape(8, 8).T.flatten()
            for w, b_ in enumerate(bs):
                mT = mask[a * 16 + b_].T  # [m, n]
                eb = np.exp(biasT[1] + mT[None])  # h, m, n
                if sel == 1 and w == 1:
                    eb = eb[:, perm][:, :, perm]
                out[i, sel, 64 * w:64 * (w + 1)] = \
                    eb.transpose(1, 0, 2).reshape(64, HEADS * 64)
    return out


def _in_maps(inputs):
    (x, qkvw_f, qkvb_in, proj_w, fc1w_f, fc1b_in, fc2_w,
     expb0, biasT, mask) = _prep(inputs)
    bf = ml_dtypes.bfloat16
    shared = {
        "qkvw": np.ascontiguousarray(qkvw_f.astype(bf)),
        "qkvb": np.ascontiguousarray(qkvb_in),
        "projw": np.ascontiguousarray(proj_w.astype(bf)),
        "fc1w": np.ascontiguousarray(fc1w_f.astype(bf)),
        "fc1b": np.ascontiguousarray(fc1b_in),
        "fc2w": np.ascontiguousarray(fc2_w.astype(bf)),
        "expb0": np.ascontiguousarray(expb0.astype(bf)),
    }
    in_maps = []
    for core in range(8):
        b, j = core // 4, core % 4
        rows = (np.arange(32 * j, 32 * j + 40)) % 128
        m = dict(shared)
        m["xin"] = np.ascontiguousarray(x[b, rows])
        m["expb1"] = np.ascontiguousarray(_expb1_core(j, biasT, mask).astype(bf))
        in_maps.append(m)
    return in_maps


def _gather(res):
    out = np.empty((B, H, W, C), np.float32)
    for core in range(8):
        b, j = core // 4, core % 4
        rows = (np.arange(32 * j + 4, 32 * j + 36)) % H
        out[b, rows] = res.results[core]["y"]
    return out.reshape(B, H * W, C)


def kernel(**inputs):
    in_maps = _in_maps(inputs)
    if "nc" not in _CACHED:
        _CACHED["nc"] = build()
    nc = _CACHED["nc"]
    res = run_bass_kernel_spmd(nc, in_maps, core_ids=list(range(8)))
    _LAST["ns"] = res.exec_time_ns
    return _gather(res)


_LAST = {"ns": None}


def last_exec_time_ns():
    return _LAST["ns"]


def run_traced(inputs):
    """Like kernel() but with trace=True; returns the full output."""
    global _TRACE
    in_maps = _in_maps(inputs)
    if "nc" not in _CACHED:
        _CACHED["nc"] = build()
    nc = _CACHED["nc"]
    res = run_bass_kernel_spmd(nc, in_maps, core_ids=list(range(8)), trace=True)
    _LAST["ns"] = res.exec_time_ns
    _TRACE = res
    return _gather(res)


# revision 14
# speedup vs baseline: 1.0164x; 1.0164x over previous
"""BasicUformerLayer (2-block Swin/Uformer stage) Trainium2 Bass kernel.

Sharding: 8 cores = (batch b in {0,1}) x (row-quarter j in {0..3}).
Core receives xin rows [32j, 32j+40) (40-row slice), computes block 0 for
all 5 bands (x1 rows [32j+4, 32j+36) kept), block 1 for the 4 shifted
bands fully inside that range, and outputs y rows [32j+4, 32j+36); the
host stitches quarters. Each shifted band is computed by exactly one
core (no duplicated block-1 bands; block-0 redundancy is 2 half-bands).

Layouts on core:
 - tokens processed in 128-token tiles = 2 windows x 64 tokens, partition
   p = w*64 + r*8 + c (window-major); groups of 4 tiles (512 tokens).
 - ALL matmuls bf16 (FWL weight loads).
 - window attention with transposed scores S^T[m,n]; exp via ACT,
   rel-pos bias multiplicative (exp(bias) precomputed), denominator via
   ones-column in V.
 - LayerNorm rstd via DVE-only Newton rsqrt; ACT runs only exp/gelu.
 - block0 -> block1 residual x1 in DRAM (bf16); NO barrier at the block
   boundary: each block-1 band's x1 rows are stored >=6 groups earlier.
 - software pipeline per group g: prep_dma(g+1) (x DMAs + LN1 stats) at
   the top of main(g); prep_xform(g+1) (Newton + LN applies + XnT
   transposes) after do_back(1); qkt(g+1) after the backs; front(g+1,0)
   after fc1 -> PE never drains at group boundaries (HAM stays warm).
 - PSUM banks: scores/AV 4 (concurrent tile-position writers must hit
   different banks unless partition-disjoint!), transposes 2 (+V),
   QK/fc1/proj/fc2 share 2.
"""

import numpy as np
import ml_dtypes

import concourse.bass as bass
from concourse import bacc
import concourse.mybir as mybir
import concourse.tile as tile
from concourse.bass_utils import run_bass_kernel_spmd
from concourse.masks import make_identity

WS = 8
HEADS = 16
HD = 32
C = 512
HID = 2048
H = W = 128
B = 2
EPS = 1e-5
F32, BF16, U32 = mybir.dt.float32, mybir.dt.bfloat16, mybir.dt.uint32
AF = mybir.ActivationFunctionType
OP = mybir.AluOpType


def _rel_pos_index(ws):
    coords = np.stack(np.meshgrid(np.arange(ws), np.arange(ws), indexing='ij')).reshape(2, -1)
    rel = (coords[:, :, None] - coords[:, None, :]).transpose(1, 2, 0)
    rel[:, :, 0] += ws - 1
    rel[:, :, 1] += ws - 1
    rel[:, :, 0] *= 2 * ws - 1
    return rel.sum(-1)


def _shift_attn_mask(H_, W_, ws, shift):
    img = np.zeros((H_, W_))
    cnt = 0
    for hs in (slice(0, -ws), slice(-ws, -shift), slice(-shift, None)):
        for wsl in (slice(0, -ws), slice(-ws, -shift), slice(-shift, None)):
            img[hs, wsl] = cnt
            cnt += 1
    mw = img.reshape(H_ // ws, ws, W_ // ws, ws).transpose(0, 2, 1, 3).reshape(-1, ws * ws)
    diff = mw[:, None, :] - mw[:, :, None]
    return np.where(diff != 0, -100.0, 0.0).astype(np.float32)  # (nW, N, N)


# ---------------------------------------------------------------- kernel build

def _rsqrt4(nc, pools, mvg, magic):
    """Batch 1/sqrt(var+eps) for 4 tiles on DVE only (no ACT table).
    mvg [128,4,2] f32 (mean,var); returns f32 view [128,4] of rstd."""
    uf = pools["small2"].tile([128, 4], F32, name="uf", tag="uf")
    nc.vector.tensor_scalar(uf, mvg[:, :, 1], EPS, None, OP.add)
    iu = pools["small2"].tile([128, 4], U32, name="iu", tag="iu")
    nc.vector.tensor_scalar(iu, uf[:].bitcast(U32), 1, None, OP.logical_shift_right)
    nc.vector.tensor_tensor(iu, magic[:], iu, OP.subtract)
    y = iu[:].bitcast(F32)
    t = pools["small2"].tile([128, 4], F32, name="nt", tag="nt")
    for _ in range(2):
        nc.vector.tensor_tensor(t, y, y, OP.mult)
        nc.vector.tensor_tensor(t, t, uf, OP.mult)
        nc.vector.tensor_scalar(t, t, -0.5, 1.5, OP.mult, OP.add)
        nc.vector.tensor_tensor(y, y, t, OP.mult)
    return y


def _transpose4(nc, pools, src_bf, dst, dst_k_slice, identity, on_act=False):
    """src [128,512] bf16 -> dst[:, k, dst_k_slice] = src chunkT (4 PE transposes)."""
    for k in range(4):
        pst = pools["ps_tp"].tile([128, 128], BF16, name="tp", tag="tp")
        nc.tensor.transpose(pst, src_bf[:, 128 * k:128 * (k + 1)], identity)
        if on_act:
            nc.scalar.activation(dst[:, k, dst_k_slice], pst[:], AF.Copy)
        else:
            nc.vector.tensor_copy(dst[:, k, dst_k_slice], pst)


def _attn_front(nc, pools, XnT, QT, KT, toff, qkvw, expb):
    """V + scores + exp + bias-mult for one 128-token tile; returns (Vp, attn)."""
    tslice = slice(toff, toff + 128)
    Vp = pools["act"].tile([128, HEADS, 64], BF16, name="Vp", tag="Vp")
    nc.vector.memset(Vp[:, :, 32:33], 1.0)
    psv = pools["ps_tp"].tile([128, 512], F32, name="psv", tag="tp")
    for k in range(4):
        nc.tensor.matmul(psv, XnT[:, k, tslice], qkvw[:, k, 1024:1536],
                         start=(k == 0), stop=(k == 3))
    nc.scalar.activation(Vp[:, :, 0:32],
                         psv[:].rearrange("p (h e) -> p h e", h=HEADS), AF.Copy)
    sb = []
    for i in range(4):
        t_ = pools["ps_s"].tile([128, 4, 64], F32, name=f"s{i}", tag=f"sa{i}")
        sb.append(t_)
    for g in range(4):
        for i in range(4):
            for w in range(2):
                nc.tensor.matmul(
                    sb[i][64 * w:64 * w + 64, g, :],
                    KT[32 * i:32 * i + 32, g, toff + 64 * w:toff + 64 * w + 64],
                    QT[32 * i:32 * i + 32, g, toff + 64 * w:toff + 64 * w + 64],
                    start=True, stop=True, tile_position=(32 * i, 64 * w))
    attn = pools["act"].tile([128, HEADS, 64], BF16, name="attn", tag="attn")
    for i in range(4):
        nc.scalar.activation(attn[:, i:HEADS:4, :], sb[i][:], AF.Exp)
    nc.vector.tensor_tensor(attn[:], attn[:], expb, OP.mult)
    return Vp, attn


def _attn_back(nc, pools, Vp, attn, O):
    """AV (+denominator) + normalize into O [128,512] bf16."""
    ab = []
    for i in range(4):
        t_ = pools["ps_s"].tile([128, 4, 64], F32, name=f"a{i}", tag=f"sa{i}")
        ab.append(t_)
    for h in range(HEADS):
        for w in range(2):
            nc.tensor.matmul(
                ab[h % 4][64 * w:64 * w + 64, h // 4, 0:33],
                attn[64 * w:64 * w + 64, h, :],
                Vp[64 * w:64 * w + 64, h, 0:33],
                start=True, stop=True, tile_position=(64 * w, 64 * w))
    rden = pools["act"].tile([128, 4, 4], F32, name="rden", tag="rden")
    Ov = O[:].rearrange("p (h e) -> p h e", h=HEADS)
    for i in range(4):
        nc.vector.reciprocal(rden[:, i], ab[i][:, :, 32])
        nc.vector.tensor_tensor(
            Ov[:, i:HEADS:4, :], ab[i][:, :, 0:32],
            rden[:, i, :, None].to_broadcast((128, 4, 32)),
            OP.mult)


def build(act=AF.Gelu):
    """Build the per-core Bacc program (same NEFF on all 8 cores)."""
    nc = bacc.Bacc("TRN2", target_bir_lowering=False, debug=False)

    xin_d = nc.dram_tensor("xin", (40, 128, C), F32, kind="ExternalInput")
    qkvw_d = nc.dram_tensor("qkvw", (2, C, 3 * C), BF16, kind="ExternalInput")
    qkvb_d = nc.dram_tensor("qkvb", (2, 128, 12), F32, kind="ExternalInput")
    projw_d = nc.dram_tensor("projw", (2, C, C), BF16, kind="ExternalInput")
    fc1w_d = nc.dram_tensor("fc1w", (2, C, HID), BF16, kind="ExternalInput")
    fc1b_d = nc.dram_tensor("fc1b", (2, 128, 16), F32, kind="ExternalInput")
    fc2w_d = nc.dram_tensor("fc2w", (2, HID, C), BF16, kind="ExternalInput")
    expb0_d = nc.dram_tensor("expb0", (128, HEADS * 64), BF16, kind="ExternalInput")
    expb1_d = nc.dram_tensor("expb1", (4, 2, 128, HEADS * 64), BF16, kind="ExternalInput")
    y_d = nc.dram_tensor("y", (32, 128, C), F32, kind="ExternalOutput")

    with tile.TileContext(nc) as tc:
        pools = {}
        import contextlib
        ctx = contextlib.ExitStack()
        with ctx:
            pools["w"] = ctx.enter_context(tc.tile_pool(name="w", bufs=1))
            pools["w2"] = ctx.enter_context(tc.tile_pool(name="w2", bufs=2))
            pools["const"] = ctx.enter_context(tc.tile_pool(name="const", bufs=1))
            pools["act"] = ctx.enter_context(tc.tile_pool(name="act", bufs=2))
            pools["x2t"] = ctx.enter_context(tc.tile_pool(name="x2t", bufs=1))
            pools["x"] = ctx.enter_context(tc.tile_pool(name="x", bufs=10))
            pools["xn"] = ctx.enter_context(tc.tile_pool(name="xn", bufs=4))
            pools["xm"] = ctx.enter_context(tc.tile_pool(name="xm", bufs=5))
            pools["h1"] = ctx.enter_context(tc.tile_pool(name="h1", bufs=1))
            pools["dram"] = ctx.enter_context(tc.tile_pool(name="dram", bufs=1, space="DRAM"))
            pools["eb"] = ctx.enter_context(tc.tile_pool(name="eb", bufs=2))
            pools["small"] = ctx.enter_context(tc.tile_pool(name="small", bufs=4))
            pools["small2"] = ctx.enter_context(tc.tile_pool(name="small2", bufs=3))
            pools["ps_tp"] = ctx.enter_context(tc.tile_pool(name="ps_tp", bufs=2, space="PSUM"))
            pools["ps_qk"] = ctx.enter_context(tc.tile_pool(name="ps_qk", bufs=2, space="PSUM"))
            pools["ps_v"] = pools["ps_qk"]  # proj/fc2 share the 2 qk banks
            pools["ps_s"] = ctx.enter_context(tc.tile_pool(name="ps_s", bufs=1, space="PSUM"))

            identity_f32 = pools["const"].tile([128, 128], F32, name="identity_f32")
            make_identity(nc, identity_f32)
            identity_b = pools["const"].tile([128, 128], BF16, name="identity")
            nc.vector.tensor_copy(identity_b, identity_f32)
            identity = identity_b[:]
            magic = pools["const"].tile([128, 4], U32, name="magic")
            nc.vector._memset_packed(magic[:], 0x5f3759df)
            expb0 = pools["const"].tile([128, HEADS, 64], BF16, name="expb0")
            nc.sync.dma_start(expb0, expb0_d[:].rearrange("p (h n) -> p h n", h=HEADS))

            # block0 -> block1 residual in DRAM (bf16). No barrier: stores of
            # the rows any block-1 band reads complete >=6 groups earlier,
            # and loads are on the gpsimd DMA queue (stores: sync queue).
            x1 = pools["dram"].tile([40, 128, C], BF16, name="x1")

            # flattened group list across both blocks
            all_groups = []
            for d in (0, 1):
                tiles = ([(k, t) for k in range(5) for t in range(8)] if d == 0
                         else [(i, t) for i in range(4) for t in range(8)])
                for gi in range(0, len(tiles), 4):
                    all_groups.append((d, tiles[gi:gi + 4]))
            NG = len(all_groups)

            wq = {}   # per-d qkv weights (loaded at prep_dma crossing)
            wm = {}   # per-d proj/fc1 weights (prefetched 2 groups early)
            wf2 = {}  # per-d fc2 weights (loaded at main crossing)
            ebs = {}  # per-stripe exp-bias tiles (block 1)
            state = {}  # gidx -> dict

            def load_wq(d):
                qkvw = pools["w2"].tile([128, 4, 3 * C], BF16, name="qkvw", tag="qkvw")
                nc.sync.dma_start(qkvw, qkvw_d[d].rearrange("(ko ki) n -> ki ko n", ki=128))
                qkvb = pools["w2"].tile([128, 12], F32, name="qkvb", tag="qkvb")
                nc.sync.dma_start(qkvb, qkvb_d[d])
                wq[d] = (qkvw, qkvb)

            def load_wm_early(d):
                projw = pools["w2"].tile([128, 4, C], BF16, name="projw", tag="projw")
                nc.sync.dma_start(projw, projw_d[d].rearrange("(ko ki) n -> ki ko n", ki=128))
                fc1w = pools["w2"].tile([128, 4, HID], BF16, name="fc1w", tag="fc1w")
                nc.sync.dma_start(fc1w, fc1w_d[d].rearrange("(ko ki) n -> ki ko n", ki=128))
                fc1b = pools["w2"].tile([128, 16], F32, name="fc1b", tag="fc1b")
                nc.sync.dma_start(fc1b, fc1b_d[d])
                wm[d] = (projw, fc1w, fc1b)

            def load_wm_late(d):
                fc2w = pools["w"].tile([128, 16, C], BF16, name="fc2w", tag="fc2w")
                nc.sync.dma_start(fc2w, fc2w_d[d].rearrange("(ko ki) n -> ki ko n", ki=128))
                wf2[d] = fc2w

            def prep_dma(gidx):
                """x DMAs (gpsimd queue) + LN1 stats for all 4 tiles of gidx."""
                d, gts = all_groups[gidx]
                if d not in wq:
                    load_wq(d)
                st_XnT = pools["act"].tile([128, 4, 512], BF16, name="XnT", tag="XnT")
                st_mvg = pools["small"].tile([128, 4, 2], F32, name="mvg", tag="mvg")
                state[gidx] = {"XnT": st_XnT, "xs": [], "mvg": st_mvg,
                               "xns": [], "fronts": []}
                st = state[gidx]
                for ti, (kk, t) in enumerate(gts):
                    if d == 1 and t == 0:
                        eb_u = pools["eb"].tile([128, HEADS, 64], BF16, name="eb_u", tag="eb_u")
                        nc.sync.dma_start(
                            eb_u, expb1_d[kk, 0].rearrange("p (h n) -> p h n", h=HEADS))
                        eb_m = pools["eb"].tile([128, HEADS, 64], BF16, name="eb_m", tag="eb_m")
                        nc.sync.dma_start(
                            eb_m, expb1_d[kk, 1].rearrange("p (h n) -> p h n", h=HEADS))
                        ebs[kk] = (eb_u, eb_m)
                    if d == 0:
                        xt = pools["x"].tile([128, C], F32, name="xt", tag="xt")
                        for w in range(2):
                            nc.sync.dma_start(
                                xt[64 * w:64 * w + 64],
                                xin_d[8 * kk:8 * kk + 8, 16 * t + 8 * w:16 * t + 8 * w + 8, :])
                    else:
                        xt = pools["x"].tile([128, C], BF16, name="xtb", tag="xt")
                        r0 = 8 * kk + 4
                        if t < 7:
                            for w in range(2):
                                nc.sync.dma_start(
                                    xt[64 * w:64 * w + 64],
                                    x1[r0:r0 + 8, 16 * t + 4 + 8 * w:16 * t + 12 + 8 * w, :])
                        else:
                            # win14 row-major; win15 col-major (p=64+8c+r)
                            nc.sync.dma_start(xt[0:64], x1[r0:r0 + 8, 116:124, :])
                            nc.sync.dma_start(
                                xt[64:96],
                                x1[r0:r0 + 8, 124:128, :].rearrange("r c e -> c r e"))
                            nc.sync.dma_start(
                                xt[96:128],
                                x1[r0:r0 + 8, 0:4, :].rearrange("r c e -> c r e"))
                    st["xs"].append(xt)
                    stats = pools["small"].tile([128, 6], F32, name="lnstats", tag="lnstats")
                    nc.vector.bn_stats(stats, xt[:])
                    nc.vector.bn_aggr(st["mvg"][:, ti], stats)

            def prep_xform(gidx):
                """Newton rsqrt + LN1 applies + XnT transposes for gidx."""
                st = state[gidx]
                rstd = _rsqrt4(nc, pools, st["mvg"], magic)
                for tj in range(4):
                    xn = pools["xn"].tile([128, C], BF16, name="xn", tag="xn")
                    nc.vector.tensor_scalar(xn, st["xs"][tj][:], st["mvg"][:, tj, 0:1],
                                            rstd[:, tj:tj + 1], OP.subtract, OP.mult)
                    st["xns"].append(xn)
                    _transpose4(nc, pools, xn, st["XnT"],
                                slice(128 * tj, 128 * tj + 128), identity, on_act=True)

            def qkt_group(gidx):
                d, _ = all_groups[gidx]
                qkvw, qkvb = wq[d]
                XnT = state[gidx]["XnT"]
                QT = pools["act"].tile([128, 4, 512], BF16, name="QT", tag="QT")
                KT = pools["act"].tile([128, 4, 512], BF16, name="KT", tag="KT")
                for qk, dst in ((0, QT), (1, KT)):
                    for g in range(4):
                        ps = pools["ps_qk"].tile([128, 512], F32, name="psqk", tag="qk")
                        for k in range(4):
                            nc.tensor.matmul(
                                ps, qkvw[:, k, 512 * qk + 128 * g: 512 * qk + 128 * (g + 1)],
                                XnT[:, k], start=(k == 0), stop=(k == 3))
                        nc.scalar.activation(dst[:, g], ps[:], AF.Identity,
                                             bias=qkvb[:, 4 * qk + g: 4 * qk + g + 1])
                state[gidx]["qkkt"] = (QT, KT)
                return QT, KT

            def do_front(gidx, ti):
                d, gts = all_groups[gidx]
                kk, t = gts[ti]
                st = state[gidx]
                QT, KT = st["qkkt"]
                qkvw, _ = wq[d]
                if d == 0:
                    expb = expb0[:]
                else:
                    expb = ebs[kk][1 if t == 7 else 0][:]
                st["fronts"].append(
                    _attn_front(nc, pools, st["XnT"], QT, KT, 128 * ti, qkvw, expb))

            def main_group(gidx):
                d, gts = all_groups[gidx]
                if d not in wm:
                    load_wm_early(d)
                if d not in wf2:
                    load_wm_late(d)
                # prefetch proj/fc1 of the next block 2 groups before the
                # crossing (their w2 slots are free; DMA overlaps compute)
                if gidx + 2 < NG:
                    d2 = all_groups[gidx + 2][0]
                    if d2 not in wm:
                        load_wm_early(d2)
                projw, fc1w, fc1b = wm[d]
                fc2w = wf2[d]
                # next group's x loads + LN1 stats first (DMA queue decoupled
                # from this group's stores; slots sized to never block here)
                if gidx + 1 < NG:
                    prep_dma(gidx + 1)
                st = state[gidx]
                xs = st["xs"]
                mvg2 = pools["small"].tile([128, 4, 2], F32, name="mvg2", tag="mvg2")
                xmids = []

                def do_back(ti):
                    Vp, attn = st["fronts"][ti]
                    O = pools["act"].tile([128, C], BF16, name="O", tag="O")
                    _attn_back(nc, pools, Vp, attn, O)
                    OT = pools["act"].tile([128, 4, 128], BF16, name="OT", tag="OT")
                    _transpose4(nc, pools, O, OT, slice(0, 128), identity)
                    psp = pools["ps_v"].tile([128, C], F32, name="psproj", tag="qk")
                    for k in range(4):
                        nc.tensor.matmul(psp, OT[:, k], projw[:, k],
                                         start=(k == 0), stop=(k == 3))
                    xmid = pools["xm"].tile([128, C], F32, name="xmid", tag="xmid")
                    nc.vector.tensor_add(xmid, psp, xs[ti])
                    xmids.append(xmid)
                    stats = pools["small"].tile([128, 6], F32, name="lnstats", tag="lnstats")
                    nc.vector.bn_stats(stats, xmid[:])
                    nc.vector.bn_aggr(mvg2[:, ti], stats)

                for ti in range(4):
                    if ti >= len(st["fronts"]):
                        do_front(gidx, ti)
                    do_back(ti)
                    # next group's LN1 transform right after back(1): its DVE
                    # chain lands mid-group, transposes/QK fill the PE queue
                    if ti == 1 and gidx + 1 < NG:
                        prep_xform(gidx + 1)
                # next group's QK (fills PE while LN2 applies run on DVE)
                if gidx + 1 < NG:
                    qkt_group(gidx + 1)
                rstd2 = _rsqrt4(nc, pools, mvg2, magic)
                Xn2T = pools["x2t"].tile([128, 4, 512], BF16, name="Xn2T", tag="Xn2T")
                for ti in range(4):
                    xn2 = pools["xn"].tile([128, C], BF16, name="xn2", tag="xn")
                    nc.vector.tensor_scalar(xn2, xmids[ti][:], mvg2[:, ti, 0:1],
                                            rstd2[:, ti:ti + 1], OP.subtract, OP.mult)
                    _transpose4(nc, pools, xn2, Xn2T,
                                slice(128 * ti, 128 * ti + 128), identity)
                # fc1 + act -> hT bf16 [128, 16, 512]
                hT = pools["h1"].tile([128, 16, 512], BF16, name="hT", tag="hT")
                for hc in range(16):
                    psf = pools["ps_qk"].tile([128, 512], F32, name="psfc1", tag="qk")
                    for k in range(4):
                        nc.tensor.matmul(psf, fc1w[:, k, 128 * hc:128 * (hc + 1)],
                                         Xn2T[:, k], start=(k == 0), stop=(k == 3))
                    nc.scalar.activation(hT[:, hc], psf, act, bias=fc1b[:, hc:hc + 1])
                # pre-emit next group's first attention front: its V/scores
                # run during fc2 and the softmax exp overlaps fc2's stream,
                # so AV is ready the moment fc2 drains
                if gidx + 1 < NG:
                    do_front(gidx + 1, 0)
                # fc2 + residual + store
                for ti, (kk, t) in enumerate(gts):
                    psf2 = pools["ps_v"].tile([128, C], F32, name="psfc2", tag="qk")
                    for hc in range(16):
                        nc.tensor.matmul(psf2, hT[:, hc, 128 * ti:128 * (ti + 1)],
                                         fc2w[:, hc], start=(hc == 0), stop=(hc == 15))
                    if d == 0:
                        ytb = pools["act"].tile([128, C], BF16, name="ytb", tag="ytb")
                        nc.vector.tensor_add(ytb, psf2, xmids[ti])
                        rlo, rhi = (4, 8) if kk == 0 else ((0, 4) if kk == 4 else (0, 8))
                        for w in range(2):
                            nc.sync.dma_start(
                                x1[8 * kk + rlo:8 * kk + rhi,
                                   16 * t + 8 * w:16 * t + 8 * w + 8, :],
                                ytb[64 * w + 8 * rlo:64 * w + 8 * rhi])
                    else:
                        yt = pools["act"].tile([128, C], F32, name="yt", tag="yt")
                        nc.vector.tensor_add(yt, psf2, xmids[ti])
                        rlo, rhi = 0, 8
                        ylo, yhi = 8 * kk, 8 * kk + 8
                        if t < 7:
                            for w in range(2):
                                nc.sync.dma_start(
                                    y_d[ylo:yhi, 16 * t + 4 + 8 * w:16 * t + 12 + 8 * w, :],
                                    yt[64 * w + 8 * rlo:64 * w + 8 * rhi])
                        else:
                            nc.sync.dma_start(y_d[ylo:yhi, 116:124, :], yt[8 * rlo:8 * rhi])
                            for c in range(4):
                                nc.sync.dma_start(y_d[ylo:yhi, 124 + c, :],
                                                  yt[64 + 8 * c + rlo:64 + 8 * c + rhi])
                                nc.sync.dma_start(y_d[ylo:yhi, c, :],
                                                  yt[96 + 8 * c + rlo:96 + 8 * c + rhi])
                del state[gidx]

            # prologue: prep + QKT of group 0
            prep_dma(0)
            prep_xform(0)
            qkt_group(0)
            for g in range(NG):
                main_group(g)
    nc.finalize()
    return nc


# ---------------------------------------------------------------- host wrapper

_CACHED = {}


def _prep(inputs):
    x = np.asarray(inputs["x"], np.float32).reshape(B, H, W, C)
    qkv_w = np.asarray(inputs["qkv_w"], np.float32)
    qkv_b = np.asarray(inputs["qkv_b"], np.float32)
    n1w = np.asarray(inputs["norm1_w"], np.float32)
    n1b = np.asarray(inputs["norm1_b"], np.float32)
    n2w = np.asarray(inputs["norm2_w"], np.float32)
    n2b = np.asarray(inputs["norm2_b"], np.float32)
    proj_w = np.asarray(inputs["proj_w"], np.float32)
    proj_b = np.asarray(inputs["proj_b"], np.float32)
    fc1_w = np.asarray(inputs["fc1_w"], np.float32)
    fc1_b = np.asarray(inputs["fc1_b"], np.float32)
    fc2_w = np.asarray(inputs["fc2_w"], np.float32)
    fc2_b = np.asarray(inputs["fc2_b"], np.float32)
    rpb = np.asarray(inputs["rpb"], np.float32)

    assert not proj_b.any() and not fc2_b.any(), "nonzero proj/fc2 bias unsupported"

    sc = HD ** -0.5
    qkvw_f = np.empty((2, C, 3 * C), np.float32)
    qkvb_f = np.empty((2, 3 * C), np.float32)
    fc1w_f = np.empty((2, C, HID), np.float32)
    fc1b_f = np.empty((2, HID), np.float32)
    for d in range(2):
        wf = qkv_w[d] * n1w[d][:, None]
        bf = qkv_b[d] + n1b[d] @ qkv_w[d]
        wf[:, :C] *= sc
        bf[:C] *= sc
        qkvw_f[d] = wf
        qkvb_f[d] = bf
        fc1w_f[d] = fc1_w[d] * n2w[d][:, None]
        fc1b_f[d] = fc1_b[d] + n2b[d] @ fc1_w[d]
    assert not qkvb_f[:, 1024:].any(), "nonzero V bias unsupported"
    qkvb_qk = qkvb_f[:, :1024].reshape(2, 8, 128).transpose(0, 2, 1)  # (2,128,8)
    qkvb_in = np.concatenate([qkvb_qk, np.zeros((2, 128, 4), np.float32)], axis=2)
    fc1b_in = fc1b_f.reshape(2, 16, 128).transpose(0, 2, 1)  # (2,128,16)

    rel = _rel_pos_index(WS)
    biasT = np.empty((2, HEADS, 64, 64), np.float32)  # [d, h, m, n]
    for d in range(2):
        bd = rpb[d][rel]  # (n, m, heads)
        biasT[d] = bd.transpose(2, 1, 0)  # h, m, n
    expb0 = np.exp(biasT[0]).transpose(1, 0, 2).reshape(64, HEADS * 64)
    expb0 = np.concatenate([expb0, expb0], 0)  # (128, 1024) both windows

    mask = _shift_attn_mask(H, W, WS, WS // 2)  # (256, 64, 64) [win, n, m]
    return (x, qkvw_f, qkvb_in, proj_w, fc1w_f, fc1b_in, fc2_w,
            expb0, biasT, mask)


def _expb1_core(j, biasT, mask):
    """(4, 2, 128, 1024) per-core block-1 exp-bias tables."""
    out = np.empty((4, 2, 128, HEADS * 64), np.float32)
    for i in range(4):
        a = (4 * j + i) % 16
        for sel in range(2):
            bs = (14, 15) if sel == 1 else (0, 1)
            perm = np.arange(64).reshape(8, 8).T.flatten()
            for w, b_ in enumerate(bs):
                mT = mask[a * 16 + b_].T  # [m, n]
                eb = np.exp(biasT[1] + mT[None])  # h, m, n
                if sel == 1 and w == 1:
                    eb = eb[:, perm][:, :, perm]
                out[i, sel, 64 * w:64 * (w + 1)] = \
                    eb.transpose(1, 0, 2).reshape(64, HEADS * 64)
    return out


def _in_maps(inputs):
    (x, qkvw_f, qkvb_in, proj_w, fc1w_f, fc1b_in, fc2_w,
     expb0, biasT, mask) = _prep(inputs)
    bf = ml_dtypes.bfloat16
    shared = {
        "qkvw": np.ascontiguousarray(qkvw_f.astype(bf)),
        "qkvb": np.ascontiguousarray(qkvb_in),
        "projw": np.ascontiguousarray(proj_w.astype(bf)),
        "fc1w": np.ascontiguousarray(fc1w_f.astype(bf)),
        "fc1b": np.ascontiguousarray(fc1b_in),
        "fc2w": np.ascontiguousarray(fc2_w.astype(bf)),
        "expb0": np.ascontiguousarray(expb0.astype(bf)),
    }
    in_maps = []
    for core in range(8):
        b, j = core // 4, core % 4
        rows = (np.arange(32 * j, 32 * j + 40)) % 128
        m = dict(shared)
        m["xin"] = np.ascontiguousarray(x[b, rows])
        m["expb1"] = np.ascontiguousarray(_expb1_core(j, biasT, mask).astype(bf))
        in_maps.append(m)
    return in_maps


def _gather(res):
    out = np.empty((B, H, W, C), np.float32)
    for core in range(8):
        b, j = core // 4, core % 4
        rows = (np.arange(32 * j + 4, 32 * j + 36)) % H
        out[b, rows] = res.results[core]["y"]
    return out.reshape(B, H * W, C)


def kernel(**inputs):
    in_maps = _in_maps(inputs)
    if "nc" not in _CACHED:
        _CACHED["nc"] = build()
    nc = _CACHED["nc"]
    res = run_bass_kernel_spmd(nc, in_maps, core_ids=list(range(8)))
    _LAST["ns"] = res.exec_time_ns
    return _gather(res)


_LAST = {"ns": None}


def last_exec_time_ns():
    return _LAST["ns"]


def run_traced(inputs):
    """Like kernel() but with trace=True; returns the full output."""
    global _TRACE
    in_maps = _in_maps(inputs)
    if "nc" not in _CACHED:
        _CACHED["nc"] = build()
    nc = _CACHED["nc"]
    res = run_bass_kernel_spmd(nc, in_maps, core_ids=list(range(8)), trace=True)
    _LAST["ns"] = res.exec_time_ns
    _TRACE = res
    return _gather(res)
